# revision 1
# baseline (speedup 1.0000x reference)
"""Trainium2 kernel for nn_Decoder_49005576847865.

Strategy (per sharding_hint): data-parallel over batch across the 8
NeuronCores (B=16 -> 2 examples/core). All masks/attention are
per-example so no cross-core communication is needed; outputs are
gathered on host.

Algorithmic restructuring vs the reference:
  * Incremental decoding with a self-attention KV cache: step t
    computes only the newest token's representations (the reference
    recomputes all t tokens every step). Exact: causal masking makes
    position i's representation independent of positions > i.
  * Cross-attention K/V are projected from encoder_output ONCE per
    layer (the reference redoes this 16x, once per decode step).
Both transforms are numerically exact restructurings.
"""

import numpy as np

NUM_HEADS = 8
NUM_LAYERS = 4
TOKEN_AMOUNT = 16
TOKEN_SIZE = 256
D_MODEL = 512
N_CORES = 8
B = 16

_compiled = None


def _pos_enc_np(T, d):
    i = np.arange(d)
    factors = 1.0 / np.power(10000.0, (2.0 * (i // 2)).astype(np.float32) / d)
    ang = np.arange(T, dtype=np.float32)[:, None] * factors
    return np.where(i % 2 == 0, np.sin(ang), np.cos(ang)).astype(np.float32)


def _build():
    import jax
    import jax.numpy as jnp

    FMAX = float(np.finfo(np.float32).max)
    pos = jnp.asarray(_pos_enc_np(TOKEN_AMOUNT, D_MODEL))

    def _ln(x, g, b, eps=1e-6):
        m = jnp.mean(x, axis=-1, keepdims=True)
        v = jnp.mean((x - m) ** 2, axis=-1, keepdims=True)
        return (x - m) / jnp.sqrt(v + eps) * g + b

    def decode(enc, enc_in, emb_W, emb_b, out_W, out_b,
               self_W, self_b, cross_W, cross_b,
               ffn_W1, ffn_b1, ffn_W2, ffn_b2, ln_g, ln_b):
        b = enc.shape[0]
        H, dh = NUM_HEADS, D_MODEL // NUM_HEADS
        # encoder padding mask (zeros for randn inputs, but computed exactly)
        pad = jnp.min((enc_in == 0).astype(jnp.float32), axis=2)  # [b, Lenc]
        # cross-attention K/V: once per layer, not once per step
        Kc, Vc = [], []
        for l in range(NUM_LAYERS):
            k = enc @ cross_W[l, 1] + cross_b[l, 1]
            v = enc @ cross_W[l, 2] + cross_b[l, 2]
            Kc.append(k.reshape(b, -1, H, dh))
            Vc.append(v.reshape(b, -1, H, dh))

        token = jnp.ones((b, TOKEN_SIZE), jnp.float32)
        Ks = [None] * NUM_LAYERS
        Vs = [None] * NUM_LAYERS
        tok_pad = jnp.zeros((b, 0), jnp.float32)  # all-zero-token key mask
        outs = []
        for t in range(TOKEN_AMOUNT):
            tp = jnp.min((token == 0).astype(jnp.float32), axis=1)  # [b]
            tok_pad = jnp.concatenate([tok_pad, tp[:, None]], axis=1)
            x = (token @ emb_W + emb_b) * jnp.sqrt(jnp.float32(D_MODEL)) + pos[t]
            for l in range(NUM_LAYERS):
                W, bb = self_W[l], self_b[l]
                q = (x @ W[0] + bb[0]).reshape(b, H, dh)
                k = (x @ W[1] + bb[1]).reshape(b, 1, H, dh)
                v = (x @ W[2] + bb[2]).reshape(b, 1, H, dh)
                Ks[l] = k if Ks[l] is None else jnp.concatenate([Ks[l], k], 1)
                Vs[l] = v if Vs[l] is None else jnp.concatenate([Vs[l], v], 1)
                lg = jnp.einsum('bhd,bkhd->bhk', q, Ks[l]) / jnp.sqrt(jnp.float32(dh))
                lg = lg - tok_pad[:, None, :] * FMAX
                w = jax.nn.softmax(lg, axis=-1)
                o = jnp.einsum('bhk,bkhd->bhd', w, Vs[l]).reshape(b, D_MODEL)
                x = _ln(x + (o @ W[3] + bb[3]), ln_g[l, 0], ln_b[l, 0])

                q = (x @ cross_W[l, 0] + cross_b[l, 0]).reshape(b, H, dh)
                lg = jnp.einsum('bhd,bkhd->bhk', q, Kc[l]) / jnp.sqrt(jnp.float32(dh))
                lg = lg - pad[:, None, :] * FMAX
                w = jax.nn.softmax(lg, axis=-1)
                o = jnp.einsum('bhk,bkhd->bhd', w, Vc[l]).reshape(b, D_MODEL)
                x = _ln(x + (o @ cross_W[l, 3] + cross_b[l, 3]), ln_g[l, 1], ln_b[l, 1])

                f = jax.nn.relu(x @ ffn_W1[l] + ffn_b1[l]) @ ffn_W2[l] + ffn_b2[l]
                x = _ln(x + f, ln_g[l, 2], ln_b[l, 2])
            token = x @ out_W + out_b
            outs.append(token)
        return jnp.stack(outs, axis=1)  # [b, 16, 256]

    n_weight_args = 14
    pmapped = jax.pmap(decode, in_axes=(0, 0) + (None,) * n_weight_args)
    jitted = jax.jit(decode)
    return pmapped, jitted


def kernel(encoder_output, encoder_input, emb_W, emb_b, out_W, out_b,
           self_W, self_b, cross_W, cross_b,
           ffn_W1, ffn_b1, ffn_W2, ffn_b2, ln_g, ln_b):
    global _compiled
    if _compiled is None:
        _compiled = _build()
    pmapped, jitted = _compiled
    weights = (emb_W, emb_b, out_W, out_b, self_W, self_b, cross_W, cross_b,
               ffn_W1, ffn_b1, ffn_W2, ffn_b2, ln_g, ln_b)
    try:
        import jax
        n_dev = min(N_CORES, jax.local_device_count())
        bl = B // n_dev
        enc_sh = np.ascontiguousarray(
            encoder_output.reshape(n_dev, bl, *encoder_output.shape[1:]))
        encin_sh = np.ascontiguousarray(
            encoder_input.reshape(n_dev, bl, *encoder_input.shape[1:]))
        out = pmapped(enc_sh, encin_sh, *weights)
        out = np.asarray(out).reshape(B, TOKEN_AMOUNT, TOKEN_SIZE)
    except Exception:
        # fallback: single-device execution of the same (validated) graph
        out = np.asarray(jitted(encoder_output, encoder_input, *weights))
    return out.astype(np.float32)

